# revision 9
# baseline (speedup 1.0000x reference)
"""Multi-head SwiGLU feed-forward (nn_MultiHeadFeedForward) Trainium2 kernel.

Math (per head h of 16, head_dim d=128, ffn f=512):
    g = x_h @ gate_w[h]      # [T,128]@[128,512]
    u = x_h @ up_w[h]
    out_h = (silu(g)*u) @ down_w[h]   # [T,512]@[512,128]

Sharding: 2 heads per core across 8 cores (no cross-core communication).
On-chip layout is feature-major ("transposed"): activations live as
[feature, token] tiles so every matmul contracts along the partition dim
without any on-chip transposes.  The host pre-transposes x into
xT[h, d, t] per core and un-transposes the output.

Steady-state engine budget per 8-tile cycle (measured): PE 10.4us,
ACT 8 silu = 8.9us, DVE 8 mul = 9.7us, plus 4 pair-copies (0.69us each).
Copies are assigned 3:1 ACT:DVE by slab quarter to balance; output is
written bf16 (host upcasts) to halve drain DMA bytes.
"""

import os
import sys

import numpy as np

for _p in ("/opt/trn_rl_repo",):
    if _p not in sys.path and os.path.isdir(_p):
        sys.path.insert(0, _p)

import concourse.bass as bass
import concourse.mybir as mybir
from concourse import bacc
import concourse.tile as tile
from concourse.bass_utils import run_bass_kernel_spmd

B, S, EMB = 4, 4096, 2048
HEADS, HD, FFN = 16, 128, 512
T = B * S                      # 16384 tokens
N_CORES = 8
HPC = HEADS // N_CORES         # heads per core = 2
TOK = 256                      # tokens per on-chip tile
NT = T // TOK                  # token tiles per head
NCH = FFN // HD                # ffn chunks of 128 = 4
SLAB = 4096                    # output slab tokens (drained per pair)

F32 = mybir.dt.float32
BF16 = mybir.dt.bfloat16
AF = mybir.ActivationFunctionType


def _build_nc():
    nc = bacc.Bacc("TRN2", target_bir_lowering=False)

    xT = nc.dram_tensor("xT", [HPC, HD, T], BF16, kind="ExternalInput")
    gw = nc.dram_tensor("gw", [HPC, HD, FFN], BF16, kind="ExternalInput")
    uw = nc.dram_tensor("uw", [HPC, HD, FFN], BF16, kind="ExternalInput")
    dw = nc.dram_tensor("dw", [HPC, FFN, HD], BF16, kind="ExternalInput")
    outT = nc.dram_tensor("outT", [HPC, HD, T], BF16, kind="ExternalOutput")

    TPS = SLAB // TOK  # tiles per slab

    with tile.TileContext(nc) as tc:
        with (
            tc.tile_pool(name="wpool", bufs=1) as wpool,
            tc.tile_pool(name="gpool", bufs=2, space="PSUM") as gpool,
            tc.tile_pool(name="upool", bufs=2, space="PSUM") as upool,
            tc.tile_pool(name="sgpool", bufs=6) as sgpool,
            tc.tile_pool(name="hpool", bufs=5) as hpool,
            tc.tile_pool(name="slabs", bufs=4) as slabs,
        ):
            # Preload the ACT silu table on a dummy tile before any data
            # arrives, so the ~1.3us ACT_TABLE_LOAD overlaps the input DMAs
            # instead of delaying the first real silu.
            warm = wpool.tile([HD, 8], F32)
            nc.gpsimd.memset(warm[:], 0.0)
            warm_o = wpool.tile([HD, 8], BF16)
            nc.scalar.activation(warm_o[:], warm[:], AF.Silu)
            # Warm the PE's HAM clock gate during the ~2.5us input-DMA wait:
            # dummy matmuls accumulate PE-busy time so the first real tiles
            # run at 2.4GHz instead of the cold 1.2GHz default.
            warm_w = wpool.tile([HD, 512], BF16)
            nc.gpsimd.memset(warm_w[:], 0.0)
            warm_ps = gpool.tile([HD, NCH * TOK], F32, name="warm_ps", tag="g")
            for _ in range(4):
                nc.tensor.matmul(
                    warm_ps[:, :512],
                    lhsT=warm_w[:, 0:HD],
                    rhs=warm_w[:],
                    start=True,
                    stop=True,
                )

            # weights + the entire x shard resident in SBUF for the kernel.
            # Issue order puts tile 0's dependencies (gw h0, first x chunk)
            # first so the PE starts as early as possible.
            gw_s = wpool.tile([HD, HPC, FFN], BF16)
            uw_s = wpool.tile([HD, HPC, FFN], BF16)
            dw_s = wpool.tile([HD, HPC, NCH, HD], BF16)
            xs_full = wpool.tile([HD, HPC, T], BF16)
            XC = 512

            # Tile 0's critical path first, in tiny DMAs: gate chunk 0 (32KB)
            # and the first 256 tokens of x (64KB) let the PE start ~2us
            # earlier than a full-weight-matrix-first order.
            nc.sync.dma_start(out=gw_s[:, 0, 0:HD], in_=gw[0, :, 0:HD])
            nc.sync.dma_start(out=xs_full[:, 0, 0:TOK], in_=xT[0, :, 0:TOK])
            nc.sync.dma_start(out=gw_s[:, 0, HD:], in_=gw[0, :, HD:])
            nc.sync.dma_start(out=xs_full[:, 0, TOK:XC], in_=xT[0, :, TOK:XC])

            def load_head(h, first):
                if not first:
                    nc.sync.dma_start(out=gw_s[:, h, :], in_=gw[h])
                nc.sync.dma_start(out=uw_s[:, h, :], in_=uw[h])
                nc.sync.dma_start(
                    out=dw_s[:, h, :, :],
                    in_=dw[h].rearrange("(c p) d -> p c d", p=HD),
                )
                for xc in range(1 if first else 0, T // XC):
                    c0 = xc * XC
                    nc.sync.dma_start(
                        out=xs_full[:, h, c0 : c0 + XC],
                        in_=xT[h, :, c0 : c0 + XC],
                    )

            load_head(0, True)
            load_head(1, False)

            # Software pipeline with a 2-tile lag on the down-proj (see
            # baseline notes): tile k's down-proj + pair copy are emitted in
            # iteration k+2; the down-proj PSUM output overlays the gate-psum
            # banks of tile k+1, keeping total PSUM usage at 8 banks.
            slab = None
            pend = []  # [(hh, slab, h, t, o_target), ...] oldest first

            def emit_down_pair(p):
                # down-proj for a PAIR of tiles: 4 matmuls of N=2*TOK reading
                # the pair's joint hh, accumulating into one full PSUM bank;
                # then one [128, 2*TOK] bf16 copy to the slab and a per-pair
                # drain DMA (waits on a single engine's copy sem).
                phh, pslab, ph, pt, ops = p  # pt = SECOND tile of the pair
                for c in range(NCH):
                    nc.tensor.matmul(
                        ops,
                        lhsT=dw_s[:, ph, c, :],
                        rhs=phh[:, c, :],
                        start=(c == 0),
                        stop=(c == NCH - 1),
                    )
                pts = pt % TPS
                dst = pslab[:, (pts - 1) * TOK : (pts + 1) * TOK]
                # 2:1 ACT:DVE copy split balances the elementwise engines at
                # ~2.67us/pair each (all-ACT makes ACT the 2.91us limiter;
                # DVE copies queue behind an in-flight mul, so keep them rare
                # and rely on the bank1-overlay slack to absorb the delay).
                if (pt // 2) % 3 == 2:
                    nc.vector.tensor_copy(dst, ops)
                else:
                    nc.scalar.copy(dst, ops)
                pt0 = pt * TOK
                nc.sync.dma_start(
                    out=outT[ph, :, pt0 - TOK : pt0 + TOK],
                    in_=dst,
                )

            tiles = [(h, t) for h in range(HPC) for t in range(NT)]
            K = len(tiles)

            def emit_gate(k):
                h, t = tiles[k]
                xs = xs_full[:, h, t * TOK : (t + 1) * TOK]
                gps = gpool.tile([HD, NCH * TOK], F32, name=f"gps_{k}", tag="g")
                for c in range(NCH):
                    nc.tensor.matmul(
                        gps[:, c * TOK : (c + 1) * TOK],
                        lhsT=gw_s[:, h, c * HD : (c + 1) * HD],
                        rhs=xs,
                        start=True,
                        stop=True,
                    )
                sg = sgpool.tile([HD, NCH * TOK], BF16, name=f"sg_{k}", tag="sg")
                nc.scalar.activation(sg[:], gps[:], AF.Silu)
                return gps, sg

            # prologue: gate+silu for tile 0
            gate_next = emit_gate(0)
            hh_pair = None
            for k in range(K):
                h, t = tiles[k]
                if t % TPS == 0:
                    slab = slabs.tile([HD, SLAB], BF16, name=f"slab_{k}", tag="slab")

                # pending pair's down-proj + copy (2-tile lag: all deps
                # retired by now)
                if k % 2 == 1 and pend:
                    emit_down_pair(pend.pop(0))

                gps, sg = gate_next
                # patch the previous pair's overlay target to THIS (even)
                # tile's gate bank 0: consumed by silu(k), recycled only by
                # gate(k+2) a full period after the pair's copy
                if k % 2 == 0 and pend:
                    pend[-1] = pend[-1][:4] + (gps[:, 2 * TOK :],)

                ups = upool.tile([HD, NCH * TOK], F32, name=f"ups_{k}", tag="u")
                xs = xs_full[:, h, t * TOK : (t + 1) * TOK]
                for c in range(NCH):
                    nc.tensor.matmul(
                        ups[:, c * TOK : (c + 1) * TOK],
                        lhsT=uw_s[:, h, c * HD : (c + 1) * HD],
                        rhs=xs,
                        start=True,
                        stop=True,
                    )
                # next tile's gate+silu ahead of this tile's mul: PE runs it
                # during the mul; silu(k+1) overlaps mul(k) on ACT
                if k + 1 < K:
                    gate_next = emit_gate(k + 1)
                if k % 2 == 0:
                    hh_pair = hpool.tile(
                        [HD, NCH, 2 * TOK], BF16, name=f"hh_{k}", tag="hh"
                    )
                half = hh_pair[:, :, (k % 2) * TOK : (k % 2 + 1) * TOK]
                nc.vector.tensor_mul(
                    half,
                    sg[:].rearrange("p (c n) -> p c n", c=NCH),
                    ups[:].rearrange("p (c n) -> p c n", c=NCH),
                )

                if k % 2 == 1:
                    # pair (k-1, k) complete; its down-proj (emitted next
                    # iteration) accumulates into THIS tile's gate bank 0,
                    # already consumed by silu(k)
                    pend.append((hh_pair, slab, h, t, gps[:, 2 * TOK :]))
            # epilogue
            for p in pend:
                emit_down_pair(p)
    nc.compile()
    return nc


def _shard_inputs(inputs):
    import ml_dtypes

    bf16 = ml_dtypes.bfloat16
    x = np.asarray(inputs["x"], dtype=np.float32)
    gw = np.asarray(inputs["gate_w"], dtype=np.float32).astype(bf16)
    uw = np.asarray(inputs["up_w"], dtype=np.float32).astype(bf16)
    dw = np.asarray(inputs["down_w"], dtype=np.float32).astype(bf16)

    xh = x.reshape(T, HEADS, HD)
    xt = np.ascontiguousarray(xh.transpose(1, 2, 0)).astype(bf16)  # [16, 128, T]

    in_maps = []
    for c in range(N_CORES):
        hs = slice(HPC * c, HPC * (c + 1))
        in_maps.append(
            {
                "xT": xt[hs],
                "gw": gw[hs],
                "uw": uw[hs],
                "dw": dw[hs],
            }
        )
    return in_maps


def run(inputs, trace=False, **spmd_kwargs):
    nc = _build_nc()
    in_maps = _shard_inputs(inputs)
    res = run_bass_kernel_spmd(
        nc, in_maps, core_ids=list(range(N_CORES)), trace=trace, **spmd_kwargs
    )
    outT = np.empty((HEADS, HD, T), dtype=np.float32)
    for c in range(N_CORES):
        outT[HPC * c : HPC * (c + 1)] = np.asarray(
            res.results[c]["outT"], dtype=np.float32
        )
    out = np.ascontiguousarray(outT.transpose(2, 0, 1)).reshape(B, S, EMB)
    return out, res


def kernel(**inputs):
    out, _ = run(inputs)
    return out


# revision 10
# speedup vs baseline: 1.0327x; 1.0327x over previous
"""Multi-head SwiGLU feed-forward (nn_MultiHeadFeedForward) Trainium2 kernel.

Math (per head h of 16, head_dim d=128, ffn f=512):
    g = x_h @ gate_w[h]      # [T,128]@[128,512]
    u = x_h @ up_w[h]
    out_h = (silu(g)*u) @ down_w[h]   # [T,512]@[512,128]

Sharding: 2 heads per core across 8 cores (no cross-core communication).
On-chip layout is feature-major ("transposed"): activations live as
[feature, token] tiles so every matmul contracts along the partition dim
without any on-chip transposes.  The host pre-transposes x into
xT[h, d, t] per core and un-transposes the output.

Steady-state engine budget per 8-tile cycle (measured): PE 10.4us,
ACT 8 silu = 8.9us, DVE 8 mul = 9.7us, plus 4 pair-copies (0.69us each).
Copies are assigned 3:1 ACT:DVE by slab quarter to balance; output is
written bf16 (host upcasts) to halve drain DMA bytes.
"""

import os
import sys

import numpy as np

for _p in ("/opt/trn_rl_repo",):
    if _p not in sys.path and os.path.isdir(_p):
        sys.path.insert(0, _p)

import concourse.bass as bass
import concourse.mybir as mybir
from concourse import bacc
import concourse.tile as tile
from concourse.bass_utils import run_bass_kernel_spmd

B, S, EMB = 4, 4096, 2048
HEADS, HD, FFN = 16, 128, 512
T = B * S                      # 16384 tokens
N_CORES = 8
HPC = HEADS // N_CORES         # heads per core = 2
TOK = 256                      # tokens per on-chip tile
NT = T // TOK                  # token tiles per head
NCH = FFN // HD                # ffn chunks of 128 = 4
SLAB = 4096                    # output slab tokens (drained per pair)

F32 = mybir.dt.float32
BF16 = mybir.dt.bfloat16
AF = mybir.ActivationFunctionType


def _build_nc():
    nc = bacc.Bacc("TRN2", target_bir_lowering=False)

    xT = nc.dram_tensor("xT", [HPC, HD, T], BF16, kind="ExternalInput")
    gw = nc.dram_tensor("gw", [HPC, HD, FFN], BF16, kind="ExternalInput")
    uw = nc.dram_tensor("uw", [HPC, HD, FFN], BF16, kind="ExternalInput")
    dw = nc.dram_tensor("dw", [HPC, FFN, HD], BF16, kind="ExternalInput")
    outT = nc.dram_tensor("outT", [HPC, HD, T], BF16, kind="ExternalOutput")

    TPS = SLAB // TOK  # tiles per slab

    with tile.TileContext(nc) as tc:
        with (
            tc.tile_pool(name="wpool", bufs=1) as wpool,
            tc.tile_pool(name="gpool", bufs=2, space="PSUM") as gpool,
            tc.tile_pool(name="upool", bufs=2, space="PSUM") as upool,
            tc.tile_pool(name="sgpool", bufs=6) as sgpool,
            tc.tile_pool(name="hpool", bufs=5) as hpool,
            tc.tile_pool(name="slabs", bufs=4) as slabs,
        ):
            # Preload the ACT silu table on a dummy tile before any data
            # arrives, so the ~1.3us ACT_TABLE_LOAD overlaps the input DMAs
            # instead of delaying the first real silu.
            warm = wpool.tile([HD, 8], F32)
            nc.gpsimd.memset(warm[:], 0.0)
            warm_o = wpool.tile([HD, 8], BF16)
            nc.scalar.activation(warm_o[:], warm[:], AF.Silu)
            # Warm the PE's HAM clock gate during the ~2.5us input-DMA wait:
            # dummy matmuls accumulate PE-busy time so the first real tiles
            # run at 2.4GHz instead of the cold 1.2GHz default.
            warm_w = wpool.tile([HD, 512], BF16)
            nc.gpsimd.memset(warm_w[:], 0.0)
            warm_ps = gpool.tile([HD, NCH * TOK], F32, name="warm_ps", tag="g")
            for _ in range(4):
                nc.tensor.matmul(
                    warm_ps[:, :512],
                    lhsT=warm_w[:, 0:HD],
                    rhs=warm_w[:],
                    start=True,
                    stop=True,
                )

            # weights + the entire x shard resident in SBUF for the kernel.
            # Issue order puts tile 0's dependencies (gw h0, first x chunk)
            # first so the PE starts as early as possible.
            gw_s = wpool.tile([HD, HPC, FFN], BF16)
            uw_s = wpool.tile([HD, HPC, FFN], BF16)
            dw_s = wpool.tile([HD, HPC, NCH, HD], BF16)
            xs_full = wpool.tile([HD, HPC, T], BF16)
            XC = 512

            # Tile 0's critical path first, in tiny DMAs: gate chunk 0 (32KB)
            # and the first 256 tokens of x (64KB) let the PE start ~2us
            # earlier than a full-weight-matrix-first order.
            nc.sync.dma_start(out=gw_s[:, 0, 0:HD], in_=gw[0, :, 0:HD])
            nc.sync.dma_start(out=xs_full[:, 0, 0:TOK], in_=xT[0, :, 0:TOK])
            nc.sync.dma_start(out=gw_s[:, 0, HD:], in_=gw[0, :, HD:])
            nc.sync.dma_start(out=xs_full[:, 0, TOK:XC], in_=xT[0, :, TOK:XC])

            def load_head(h, first):
                if not first:
                    nc.sync.dma_start(out=gw_s[:, h, :], in_=gw[h])
                nc.sync.dma_start(out=uw_s[:, h, :], in_=uw[h])
                nc.sync.dma_start(
                    out=dw_s[:, h, :, :],
                    in_=dw[h].rearrange("(c p) d -> p c d", p=HD),
                )
                for xc in range(1 if first else 0, T // XC):
                    c0 = xc * XC
                    nc.sync.dma_start(
                        out=xs_full[:, h, c0 : c0 + XC],
                        in_=xT[h, :, c0 : c0 + XC],
                    )

            load_head(0, True)
            load_head(1, False)

            # Software pipeline with a 2-tile lag on the down-proj (see
            # baseline notes): tile k's down-proj + pair copy are emitted in
            # iteration k+2; the down-proj PSUM output overlays the gate-psum
            # banks of tile k+1, keeping total PSUM usage at 8 banks.
            slab = None
            pend = []  # [(hh, slab, h, t, o_target), ...] oldest first

            def emit_down_pair(p):
                # down-proj for a PAIR of tiles: 4 matmuls of N=2*TOK reading
                # the pair's joint hh, accumulating into one full PSUM bank;
                # then one [128, 2*TOK] bf16 copy to the slab and a per-pair
                # drain DMA (waits on a single engine's copy sem).
                phh, pslab, ph, pt, ops = p  # pt = SECOND tile of the pair
                for c in range(NCH):
                    nc.tensor.matmul(
                        ops,
                        lhsT=dw_s[:, ph, c, :],
                        rhs=phh[:, c, :],
                        start=(c == 0),
                        stop=(c == NCH - 1),
                    )
                pts = pt % TPS
                dst = pslab[:, (pts - 1) * TOK : (pts + 1) * TOK]
                # All copies on ACT: a DVE-assigned copy queues behind the
                # in-flight mul and lands ~500ns late, stalling gate(k+2) on
                # the overlaid bank (measured as periodic 509ns PE gaps).
                nc.scalar.copy(dst, ops)
                pt0 = pt * TOK
                nc.sync.dma_start(
                    out=outT[ph, :, pt0 - TOK : pt0 + TOK],
                    in_=dst,
                )

            tiles = [(h, t) for h in range(HPC) for t in range(NT)]
            K = len(tiles)

            def emit_gate(k):
                h, t = tiles[k]
                xs = xs_full[:, h, t * TOK : (t + 1) * TOK]
                gps = gpool.tile([HD, NCH * TOK], F32, name=f"gps_{k}", tag="g")
                for c in range(NCH):
                    nc.tensor.matmul(
                        gps[:, c * TOK : (c + 1) * TOK],
                        lhsT=gw_s[:, h, c * HD : (c + 1) * HD],
                        rhs=xs,
                        start=True,
                        stop=True,
                    )
                sg = sgpool.tile([HD, NCH * TOK], BF16, name=f"sg_{k}", tag="sg")
                nc.scalar.activation(sg[:], gps[:], AF.Silu)
                return gps, sg

            # prologue: gate+silu for tile 0
            gate_next = emit_gate(0)
            hh_pair = None
            for k in range(K):
                h, t = tiles[k]
                if t % TPS == 0:
                    slab = slabs.tile([HD, SLAB], BF16, name=f"slab_{k}", tag="slab")

                # pending pair's down-proj + copy (2-tile lag: all deps
                # retired by now)
                if k % 2 == 1 and pend:
                    emit_down_pair(pend.pop(0))

                gps, sg = gate_next
                # patch the previous pair's overlay target to THIS (even)
                # tile's gate bank 0: consumed by silu(k), recycled only by
                # gate(k+2) a full period after the pair's copy
                if k % 2 == 0 and pend:
                    pend[-1] = pend[-1][:4] + (gps[:, 2 * TOK :],)

                ups = upool.tile([HD, NCH * TOK], F32, name=f"ups_{k}", tag="u")
                xs = xs_full[:, h, t * TOK : (t + 1) * TOK]
                for c in range(NCH):
                    nc.tensor.matmul(
                        ups[:, c * TOK : (c + 1) * TOK],
                        lhsT=uw_s[:, h, c * HD : (c + 1) * HD],
                        rhs=xs,
                        start=True,
                        stop=True,
                    )
                # next tile's gate+silu ahead of this tile's mul: PE runs it
                # during the mul; silu(k+1) overlaps mul(k) on ACT
                if k + 1 < K:
                    gate_next = emit_gate(k + 1)
                if k % 2 == 0:
                    hh_pair = hpool.tile(
                        [HD, NCH, 2 * TOK], BF16, name=f"hh_{k}", tag="hh"
                    )
                half = hh_pair[:, :, (k % 2) * TOK : (k % 2 + 1) * TOK]
                nc.vector.tensor_mul(
                    half,
                    sg[:].rearrange("p (c n) -> p c n", c=NCH),
                    ups[:].rearrange("p (c n) -> p c n", c=NCH),
                )

                if k % 2 == 1:
                    # pair (k-1, k) complete; its down-proj (emitted next
                    # iteration) accumulates into THIS tile's gate bank 0,
                    # already consumed by silu(k)
                    pend.append((hh_pair, slab, h, t, gps[:, 2 * TOK :]))
            # epilogue
            for p in pend:
                emit_down_pair(p)
    nc.compile()
    return nc


def _shard_inputs(inputs):
    import ml_dtypes

    bf16 = ml_dtypes.bfloat16
    x = np.asarray(inputs["x"], dtype=np.float32)
    gw = np.asarray(inputs["gate_w"], dtype=np.float32).astype(bf16)
    uw = np.asarray(inputs["up_w"], dtype=np.float32).astype(bf16)
    dw = np.asarray(inputs["down_w"], dtype=np.float32).astype(bf16)

    xh = x.reshape(T, HEADS, HD)
    xt = np.ascontiguousarray(xh.transpose(1, 2, 0)).astype(bf16)  # [16, 128, T]

    in_maps = []
    for c in range(N_CORES):
        hs = slice(HPC * c, HPC * (c + 1))
        in_maps.append(
            {
                "xT": xt[hs],
                "gw": gw[hs],
                "uw": uw[hs],
                "dw": dw[hs],
            }
        )
    return in_maps


def run(inputs, trace=False, **spmd_kwargs):
    nc = _build_nc()
    in_maps = _shard_inputs(inputs)
    res = run_bass_kernel_spmd(
        nc, in_maps, core_ids=list(range(N_CORES)), trace=trace, **spmd_kwargs
    )
    outT = np.empty((HEADS, HD, T), dtype=np.float32)
    for c in range(N_CORES):
        outT[HPC * c : HPC * (c + 1)] = np.asarray(
            res.results[c]["outT"], dtype=np.float32
        )
    out = np.ascontiguousarray(outT.transpose(2, 0, 1)).reshape(B, S, EMB)
    return out, res


def kernel(**inputs):
    out, _ = run(inputs)
    return out
